# revision 48
# baseline (speedup 1.0000x reference)
"""Causal multi-head attention (B=2, S=2048, D=1024, H=16, hd=64) on 8 trn2 cores.

Sharding: core c handles batch b = c//4 and head group hg = c%4 (4 heads each).
Each core computes its Q/K/V shard (tensor-parallel columns of W_qkv), causal
attention for its 4 heads with scores held transposed ([s_k, s_q] so the PV
matmul needs no on-chip transposes), and a partial output projection over its
256 rows of W_proj. The host sums the 4 partials per batch and adds the exact
bias terms (softmax rows sum to 1, so attn@(V + 1 bv^T) = attn@V + bv^T; the
b_qkv V-slice and b_proj are applied on the host).

Schedule: input DMAs are split across four engine issue queues so the first
projection matmul starts ~4us in. Attention is software-pipelined: the PV
matmuls for step ki are emitted one iteration after the scores for ki, so the
exp (ACT) -> mask (Pool) latency never stalls the PE. The softmax divide is
done per 512-column PSUM bank as soon as that bank's accumulation stops,
which lets the output projection of the last q-ranges weave into the final
attention chain instead of trailing it. PSUM drains stay on DVE (gpsimd has
no PSUM port); SBUF-only work (causal mask, denominator broadcast, V spread)
runs on the otherwise-idle gpsimd.
"""

import numpy as np
import ml_dtypes
from contextlib import ExitStack

B, S, D, H = 2, 2048, 1024, 16
HD = 64
NCORES = 8
FPC = 256  # features per core (4 heads x 64)

_CACHE = {}


def _build():
    import concourse.bacc as bacc
    import concourse.tile as tile
    import concourse.mybir as mybir

    f32 = mybir.dt.float32
    bf16 = mybir.dt.bfloat16

    nc = bacc.Bacc("TRN2", target_bir_lowering=False, debug=False, num_devices=NCORES)

    xT = nc.dram_tensor("xT", [D, S], bf16, kind="ExternalInput").ap()
    wq = nc.dram_tensor("wq", [D, FPC], bf16, kind="ExternalInput").ap()
    wk = nc.dram_tensor("wk", [D, FPC], bf16, kind="ExternalInput").ap()
    wv = nc.dram_tensor("wv", [D, FPC], bf16, kind="ExternalInput").ap()
    wp = nc.dram_tensor("wp", [FPC, D], bf16, kind="ExternalInput").ap()
    bqk = nc.dram_tensor("bqk", [128, 4], f32, kind="ExternalInput").ap()
    maskT = nc.dram_tensor("maskT", [128, 128], bf16, kind="ExternalInput").ap()
    out = nc.dram_tensor("out", [S, D], bf16, kind="ExternalOutput").ap()

    with tile.TileContext(nc) as tc:
        with ExitStack() as ctx:
            _body(ctx, tc, mybir, out, xT, wq, wk, wv, wp, bqk, maskT)

    nc.compile()
    return nc


def _body(ctx, tc, mybir, out, xT, wq, wk, wv, wp, bqk, maskT):
    nc = tc.nc
    f32 = mybir.dt.float32
    bf16 = mybir.dt.bfloat16
    AF = mybir.ActivationFunctionType
    NK = D // 128   # 8 contraction tiles for qkv/proj-input dim
    NS = S // 128   # 16 sequence tiles

    sb = ctx.enter_context(tc.tile_pool(name="sb", bufs=1))

    xt_a = sb.tile([128, NK * S], bf16, name="xta", tag="xta")
    xt_t = [xt_a[:, k * S:(k + 1) * S] for k in range(NK)]
    wq_a = sb.tile([128, NK * FPC], bf16, name="wqa", tag="wqa")
    wq_t = [wq_a[:, k * FPC:(k + 1) * FPC] for k in range(NK)]
    wk_a = sb.tile([128, NK * FPC], bf16, name="wka", tag="wka")
    wk_t = [wk_a[:, k * FPC:(k + 1) * FPC] for k in range(NK)]
    wv_a = sb.tile([128, NK * FPC], bf16, name="wva", tag="wva")
    wv_t = [wv_a[:, k * FPC:(k + 1) * FPC] for k in range(NK)]
    wp_a = sb.tile([128, 2 * D], bf16, name="wpa", tag="wpa")
    wp_t = [wp_a[:, k * D:(k + 1) * D] for k in range(2)]
    qt_t = [sb.tile([128, S], bf16, name=f"qtt{f}", tag=f"qtt{f}") for f in range(2)]
    kt_t = [sb.tile([128, S], bf16, name=f"ktt{f}", tag=f"ktt{f}") for f in range(2)]
    v_t = [sb.tile([128, 4 * 65], bf16, name=f"vt{s}", tag=f"vt{s}") for s in range(NS)]
    ot_t = [sb.tile([128, S], bf16, name=f"ott{f}", tag=f"ott{f}") for f in range(2)]
    bqk_t = sb.tile([128, 4], f32, name="bqkt", tag="bqkt")
    mask_t = sb.tile([128, 128], bf16, name="maskt", tag="maskt")

    p_pool = ctx.enter_context(tc.tile_pool(name="pp", bufs=1))
    vs_pool = ctx.enter_context(tc.tile_pool(name="vsp", bufs=2))
    rc_pool = ctx.enter_context(tc.tile_pool(name="rcp", bufs=4))
    oo_pool = ctx.enter_context(tc.tile_pool(name="oop", bufs=2))

    # ---- input DMAs split across four issue queues; each queue's order puts
    # the first attention pair's dependencies first.
    xt3 = xt_a.rearrange("p (k s) -> p k s", k=NK)
    xs3 = xT.rearrange("(k p) s -> p k s", p=128)
    # DMA issue is only possible from SP (sync), Activation (scalar), gpsimd.
    # The critical path for the first matmuls (wq, then the x first-half
    # k-chunks in arrival order) rides the fastest-starting queue (sync).
    # gpsimd's DGE pays a ~6us ucode load, so it only gets late-needed data.
    # The critical prefix is split across queues, and x arrives in s-major
    # 512-column chunks so the sp-major qkt matmuls chase the stream.
    nc.sync.dma_start(wq_a.rearrange("p (k f) -> p k f", k=NK),
                      wq.rearrange("(k p) f -> p k f", p=128))
    nc.sync.dma_start(bqk_t[:], bqk[:])
    nc.sync.dma_start(xt3[:, :, 0:512], xs3[:, :, 0:512])
    # scalar: wk (2nd qkt group), mask (first exp), then second-half x
    nc.scalar.dma_start(wk_a.rearrange("p (k f) -> p k f", k=NK),
                        wk.rearrange("(k p) f -> p k f", p=128))
    nc.scalar.dma_start(mask_t[:], maskT[:])
    nc.scalar.dma_start(xt3[:, :, 1024:1536], xs3[:, :, 1024:1536])
    nc.scalar.dma_start(xt3[:, :, 1536:2048], xs3[:, :, 1536:2048])
    # gpsimd (DGE starts ~6us late): wv, then x columns 512:1024 -- taking
    # the second s-chunk off sync's back halves its serial critical path
    nc.gpsimd.dma_start(wv_a.rearrange("p (k f) -> p k f", k=NK),
                        wv.rearrange("(k p) f -> p k f", p=128))
    nc.gpsimd.dma_start(xt3[:, :, 512:1024], xs3[:, :, 512:1024])
    nc.gpsimd.dma_start(wp_a.rearrange("p (k f) -> p k f", k=2),
                        wp.rearrange("(k p) f -> p k f", p=128))

    # PSUM: "sc" ring (scores + all filler groups) 2 x [128,1024] = 4 banks,
    # "pv" accumulators 2 x [128,1024] = 4 banks.
    scp = ctx.enter_context(tc.tile_pool(name="ps_sc", bufs=2, space="PSUM"))
    pvp = ctx.enter_context(tc.tile_pool(name="ps_pv", bufs=2, space="PSUM"))

    def qkt_group(dst, w_t, bcol, f, c2):
        """One [128,1024] accumulation group of the Q^T/K^T projection.
        sp-major so the first 512 columns complete (and drain) while the
        next x s-chunk is still arriving."""
        ps = scp.tile([128, 1024], f32, name="sc", tag="sc", bufs=2)
        for sp in range(2):
            for k in range(NK):
                nc.tensor.matmul(
                    ps[:, sp * 512:(sp + 1) * 512],
                    w_t[k][:, f * 128:(f + 1) * 128],
                    xt_t[k][:, c2 * 1024 + sp * 512: c2 * 1024 + (sp + 1) * 512],
                    start=(k == 0), stop=(k == NK - 1),
                )
            nc.vector.tensor_scalar_add(
                dst[f][:, c2 * 1024 + sp * 512: c2 * 1024 + (sp + 1) * 512],
                ps[:, sp * 512:(sp + 1) * 512],
                bqk_t[:, bcol + f: bcol + f + 1],
            )

    def v_group(s):
        psv = scp.tile([128, FPC], f32, name="sc", tag="sc", bufs=2)
        for k in range(NK):
            nc.tensor.matmul(
                psv[:],
                xt_t[k][:, s * 128:(s + 1) * 128],
                wv_t[k][:],
                start=(k == 0), stop=(k == NK - 1),
            )
        # NOTE: bulk work on gpsimd (Q7) triggers the power throttle that
        # halves PE duty -- keep these on DVE.
        v3 = v_t[s].rearrange("p (h c) -> p h c", h=4)
        nc.vector.tensor_copy(v3[:, :, 0:64],
                              psv.rearrange("p (h c) -> p h c", h=4)[:])
        nc.vector.memset(v3[:, :, 64:65], 1.0)

    class AttnUnit:
        """Causal attention for head h over queries [half*1024, +1024)."""

        def __init__(self, h, half, dcp_on_act=False, fine_tail=False):
            self.h, self.half = h, half
            self.hp, self.hh = h // 2, h % 2
            self.r0 = self.hh * 64
            self.q0 = half * 1024
            self.ki_n = NS // 2 * (half + 1)
            self.dcp_on_act = dcp_on_act
            # pv accumulation regions are bank-aligned (512 cols); two
            # regions must never share a psum bank while one still
            # accumulates. fine_tail only splits the *finish* of the last
            # bank into 256-col chunks (safe: emitted after the full stop)
            # so the first projection of those columns starts sooner.
            self.pv_bounds = [0, 512, 1024]
            kf0 = min(self.ki_n - 1, (self.q0 + 511) // 128)
            last = self.ki_n - 1
            if fine_tail:
                self.fin = [(0, 512, kf0), (512, 768, last), (768, 1024, last)]
            else:
                self.fin = [(0, 512, kf0), (512, 1024, last)]
            self.pv = pvp.tile([128, 1024], f32, name="pv", tag="pv", bufs=2)
            self.P = {}
            self.spans = {}

        def emit_scores(self, ki):
            q0, r0 = self.q0, self.r0
            qt, kt = qt_t[self.hp], kt_t[self.hp]
            qs = max(ki * 128, q0)   # first unmasked q for this k block
            a0 = qs - q0             # local col offset in the 1024 tile
            diag = ki * 128 >= q0    # diagonal block lives in this half
            spans = [(a0, 512), (512, 1024)] if a0 < 512 else [(a0, 1024)]
            self.spans[ki] = (a0, diag)
            sc = scp.tile([128, 1024], f32, name="sc", tag="sc", bufs=2)
            for (a, b) in spans:
                nc.tensor.matmul(
                    sc[:, a:b],
                    kt[r0:r0 + 64, ki * 128:(ki + 1) * 128],
                    qt[r0:r0 + 64, q0 + a:q0 + b],
                    start=True, stop=True,
                )
            self.sc = sc

        def emit_exp(self, ki):
            a0, diag = self.spans[ki]
            P = p_pool.tile([128, 1024], bf16, name="P", tag="P", bufs=8)
            nc.scalar.activation(P[:, a0:1024], self.sc[:, a0:1024], AF.Exp,
                                 scale=float(HD) ** -0.5)
            if diag:  # causal mask on the diagonal block
                nc.vector.tensor_mul(P[:, a0:a0 + 128],
                                     P[:, a0:a0 + 128], mask_t[:])
            self.P[ki] = P

        def emit_pv(self, ki):
            a0, _ = self.spans[ki]
            P = self.P.pop(ki)
            # subdivide [a0,1024) at the pv region bounds
            for i in range(len(self.pv_bounds) - 1):
                a = max(a0, self.pv_bounds[i])
                b = self.pv_bounds[i + 1]
                if a >= b:
                    continue
                last_ki = min(self.ki_n - 1, (self.q0 + b - 1) // 128)
                nc.tensor.matmul(
                    self.pv[0:65, a:b],
                    v_t[ki][:, self.h * 65:self.h * 65 + 65],
                    P[:, a:b],
                    start=(ki == 0), stop=(ki == last_ki),
                )

        def finish_span(self, a, b):
            """Divide pv rows by the denominator row for columns [a,b)."""
            pv = self.pv
            w = b - a
            dcp = rc_pool.tile([1, 512], f32, name="dcp", tag="dcp", bufs=4)
            if self.dcp_on_act:
                nc.scalar.copy(dcp[:, 0:w], pv[64:65, a:b])
            else:
                nc.vector.tensor_copy(dcp[:, 0:w], pv[64:65, a:b])
            rcp = rc_pool.tile([1, 512], f32, name="rcp", tag="rcp", bufs=4)
            nc.vector.reciprocal_approx_fast(rcp[:, 0:w], dcp[:, 0:w])
            rbc = rc_pool.tile([64, 512], f32, name="rbc", tag="rbc", bufs=4)
            nc.gpsimd.partition_broadcast(rbc[:, 0:w], rcp[:, 0:w], channels=64)
            nc.vector.tensor_mul(
                ot_t[self.hp][self.r0:self.r0 + 64, self.q0 + a:self.q0 + b],
                pv[0:64, a:b], rbc[:, 0:w],
            )

    def attn_pair(ha, hb, half, fillers=(), dcp_on_act=False, fine_tail=False):
        """Two heads, software-pipelined: scores(t) and pv(t-1) per iteration
        so the exp->mask latency is hidden. fillers[t] is a list of thunks
        emitting independent PE work at the end of iteration t."""
        ua = AttnUnit(ha, half, dcp_on_act, fine_tail)
        ub = AttnUnit(hb, half, dcp_on_act, fine_tail)
        n = ua.ki_n
        # two-iteration lag between scores and pv: the exp (ACT) -> mask
        # (DVE) chain gets ~2 iterations of slack, so DVE bursts (finish
        # spans) cannot stall the PE's pv matmuls
        for t in range(n + 2):
            if t < n:
                ua.emit_scores(t)
                ub.emit_scores(t)
                ua.emit_exp(t)
                ub.emit_exp(t)
            if t >= 2:
                ua.emit_pv(t - 2)
                ub.emit_pv(t - 2)
                for (a, b, kf) in ua.fin:
                    if t - 2 == kf:
                        ua.finish_span(a, b)
                        ub.finish_span(a, b)
            if t < len(fillers):
                for fn in fillers[t]:
                    fn()

    def proj_group(s):
        pj = scp.tile([128, 1024], f32, name="sc", tag="sc", bufs=2)
        for nh in range(2):
            for k2 in range(2):
                nc.tensor.matmul(
                    pj[:, nh * 512:(nh + 1) * 512],
                    ot_t[k2][:, s * 128:(s + 1) * 128],
                    wp_t[k2][:, nh * 512:(nh + 1) * 512],
                    start=(k2 == 0), stop=(k2 == 1),
                )
        oo = oo_pool.tile([128, D], bf16, name="oo", tag="oo", bufs=3)
        # drain on ACT only in the true tail where it has gone idle; putting
        # psum-dependent drains on ACT mid-kernel blocks the in-order exp
        # stream and stalls the whole attention pipeline
        if s >= 12:
            nc.scalar.copy(oo[:], pj[:])
        else:
            nc.vector.tensor_copy(oo[:], pj[:])
        # out DMAs all ride sync: the gpsimd queue is clogged by the final
        # partition_broadcasts right when the last groups drain
        nc.sync.dma_start(out[s * 128:(s + 1) * 128, :], oo[:])

    from functools import partial

    def qkt_half(dst, w_t, bcol, f, c2, sp):
        """512-column half of a qkt group -- prelude granularity that chases
        the arriving x s-chunks."""
        ps = scp.tile([128, 512], f32, name="sc", tag="sc", bufs=2)
        for k in range(NK):
            nc.tensor.matmul(
                ps[:],
                w_t[k][:, f * 128:(f + 1) * 128],
                xt_t[k][:, c2 * 1024 + sp * 512: c2 * 1024 + (sp + 1) * 512],
                start=(k == 0), stop=(k == NK - 1),
            )
        nc.vector.tensor_scalar_add(
            dst[f][:, c2 * 1024 + sp * 512: c2 * 1024 + (sp + 1) * 512],
            ps[:], bqk_t[:, bcol + f: bcol + f + 1],
        )

    # Prelude: exactly what pair (0,1,0) needs to start, in x-arrival order.
    qkt_half(qt_t, wq_t, 0, 0, 0, 0)
    qkt_half(kt_t, wk_t, 2, 0, 0, 0)
    v_group(0)
    qkt_half(qt_t, wq_t, 0, 0, 0, 1)
    qkt_half(kt_t, wk_t, 2, 0, 0, 1)
    v_group(1)

    attn_pair(0, 1, 0, fillers=[
        [partial(v_group, 2)],
        [partial(v_group, 3)],
        [partial(v_group, 4)],
        [partial(v_group, 5)],
        [partial(v_group, 6)],
        [partial(v_group, 7)],
        [partial(qkt_group, qt_t, wq_t, 0, 1, 0)],
        [partial(qkt_group, kt_t, wk_t, 2, 1, 0)],
        [],
    ])
    attn_pair(2, 3, 0, fillers=[
        [partial(qkt_group, qt_t, wq_t, 0, 0, 1)],
        [partial(qkt_group, kt_t, wk_t, 2, 0, 1)],
        [partial(v_group, 8)],
        [],
        [partial(v_group, 9)],
        [],
        [partial(v_group, 10)],
        [],
        [],
    ])
    attn_pair(0, 1, 1, fillers=[
        [partial(qkt_group, qt_t, wq_t, 0, 1, 1)],
        [],
        [partial(qkt_group, kt_t, wk_t, 2, 1, 1)],
        [],
        [partial(v_group, 11)],
        [],
        [partial(v_group, 12)],
        [],
        [partial(v_group, 13)],
        [partial(v_group, 14)],
        [partial(v_group, 15)],
        [], [], [],
        [partial(proj_group, 0)],
        [],
        [partial(proj_group, 1)],
    ])
    attn_pair(2, 3, 1, dcp_on_act=True, fine_tail=True, fillers=[
        [partial(proj_group, 2)],
        [],
        [partial(proj_group, 3)],
        [],
        [partial(proj_group, 4)],
        [],
        [partial(proj_group, 5)],
        [],
        [partial(proj_group, 6)],
        [],
        [partial(proj_group, 7)],
        [], [], [],
        [partial(proj_group, 8)],
        [partial(proj_group, 9)],
        [partial(proj_group, 10)],
        [partial(proj_group, 11)],
    ])
    for s in range(12, NS):
        proj_group(s)


def _in_maps(x, W_qkv, b_qkv, W_proj):
    bf = ml_dtypes.bfloat16
    maps = []
    # multiplicative causal mask for the transposed diag block: keep k<=q
    mask = np.triu(np.ones((128, 128), np.float32)).astype(bf)
    for core in range(NCORES):
        b, hg = core // 4, core % 4
        cs = slice(hg * FPC, (hg + 1) * FPC)
        bq = b_qkv[cs].astype(np.float32)
        bk = b_qkv[D + hg * FPC: D + (hg + 1) * FPC].astype(np.float32)
        maps.append({
            "xT": np.ascontiguousarray(x[b].T).astype(bf),
            "wq": np.ascontiguousarray(W_qkv[:, cs]).astype(bf),
            "wk": np.ascontiguousarray(W_qkv[:, D + hg * FPC: D + (hg + 1) * FPC]).astype(bf),
            "wv": np.ascontiguousarray(W_qkv[:, 2 * D + hg * FPC: 2 * D + (hg + 1) * FPC]).astype(bf),
            "wp": np.ascontiguousarray(W_proj[hg * FPC:(hg + 1) * FPC, :]).astype(bf),
            "bqk": np.ascontiguousarray(
                np.stack([bq[0:128], bq[128:256], bk[0:128], bk[128:256]], axis=1)),
            "maskT": mask,
        })
    return maps


def get_nc():
    if "nc" not in _CACHE:
        _CACHE["nc"] = _build()
    return _CACHE["nc"]


def _postprocess(partials, b_qkv, W_proj, b_proj):
    out = np.zeros((B, S, D), np.float32)
    for core in range(NCORES):
        out[core // 4] += np.asarray(partials[core], np.float32)
    bv = np.asarray(b_qkv, np.float32)[2 * D:3 * D]
    out += bv @ np.asarray(W_proj, np.float32) + np.asarray(b_proj, np.float32)
    return out


def kernel(x, W_qkv, b_qkv, W_proj, b_proj, _trace=False):
    from concourse.bass_utils import run_bass_kernel_spmd

    x = np.asarray(x, np.float32)
    W_qkv = np.asarray(W_qkv, np.float32)
    b_qkv = np.asarray(b_qkv, np.float32)
    W_proj = np.asarray(W_proj, np.float32)
    b_proj = np.asarray(b_proj, np.float32)

    nc = get_nc()
    maps = _in_maps(x, W_qkv, b_qkv, W_proj)
    res = run_bass_kernel_spmd(nc, maps, list(range(NCORES)), trace=_trace)
    _CACHE["last_result"] = res
    partials = [res.results[c]["out"] for c in range(NCORES)]
    return _postprocess(partials, b_qkv, W_proj, b_proj)


# revision 53
# speedup vs baseline: 1.0396x; 1.0396x over previous
"""Causal multi-head attention (B=2, S=2048, D=1024, H=16, hd=64) on 8 trn2 cores.

Sharding: core c handles batch b = c//4 and head group hg = c%4 (4 heads each).
Each core computes its Q/K/V shard (tensor-parallel columns of W_qkv), causal
attention for its 4 heads with scores held transposed ([s_k, s_q] so the PV
matmul needs no on-chip transposes), and a partial output projection over its
256 rows of W_proj. The host sums the 4 partials per batch and adds the exact
bias terms (softmax rows sum to 1, so attn@(V + 1 bv^T) = attn@V + bv^T; the
b_qkv V-slice and b_proj are applied on the host).

Schedule: input DMAs are split across four engine issue queues so the first
projection matmul starts ~4us in. Attention is software-pipelined: the PV
matmuls for step ki are emitted one iteration after the scores for ki, so the
exp (ACT) -> mask (Pool) latency never stalls the PE. The softmax divide is
done per 512-column PSUM bank as soon as that bank's accumulation stops,
which lets the output projection of the last q-ranges weave into the final
attention chain instead of trailing it. PSUM drains stay on DVE (gpsimd has
no PSUM port); SBUF-only work (causal mask, denominator broadcast, V spread)
runs on the otherwise-idle gpsimd.
"""

import numpy as np
import ml_dtypes
from contextlib import ExitStack

B, S, D, H = 2, 2048, 1024, 16
HD = 64
NCORES = 8
FPC = 256  # features per core (4 heads x 64)

_CACHE = {}


def _build():
    import concourse.bacc as bacc
    import concourse.tile as tile
    import concourse.mybir as mybir

    f32 = mybir.dt.float32
    bf16 = mybir.dt.bfloat16

    nc = bacc.Bacc("TRN2", target_bir_lowering=False, debug=False, num_devices=NCORES)

    xT = nc.dram_tensor("xT", [D, S], bf16, kind="ExternalInput").ap()
    wq = nc.dram_tensor("wq", [D, FPC], bf16, kind="ExternalInput").ap()
    wk = nc.dram_tensor("wk", [D, FPC], bf16, kind="ExternalInput").ap()
    wv = nc.dram_tensor("wv", [D, FPC], bf16, kind="ExternalInput").ap()
    wp = nc.dram_tensor("wp", [FPC, D], bf16, kind="ExternalInput").ap()
    bqk = nc.dram_tensor("bqk", [128, 4], f32, kind="ExternalInput").ap()
    maskT = nc.dram_tensor("maskT", [128, 128], bf16, kind="ExternalInput").ap()
    out = nc.dram_tensor("out", [S, D], bf16, kind="ExternalOutput").ap()

    with tile.TileContext(nc) as tc:
        with ExitStack() as ctx:
            _body(ctx, tc, mybir, out, xT, wq, wk, wv, wp, bqk, maskT)

    nc.compile()
    return nc


def _body(ctx, tc, mybir, out, xT, wq, wk, wv, wp, bqk, maskT):
    nc = tc.nc
    f32 = mybir.dt.float32
    bf16 = mybir.dt.bfloat16
    AF = mybir.ActivationFunctionType
    NK = D // 128   # 8 contraction tiles for qkv/proj-input dim
    NS = S // 128   # 16 sequence tiles

    sb = ctx.enter_context(tc.tile_pool(name="sb", bufs=1))

    xt_a = sb.tile([128, NK * S], bf16, name="xta", tag="xta")
    xt_t = [xt_a[:, k * S:(k + 1) * S] for k in range(NK)]
    wq_a = sb.tile([128, NK * FPC], bf16, name="wqa", tag="wqa")
    wq_t = [wq_a[:, k * FPC:(k + 1) * FPC] for k in range(NK)]
    wk_a = sb.tile([128, NK * FPC], bf16, name="wka", tag="wka")
    wk_t = [wk_a[:, k * FPC:(k + 1) * FPC] for k in range(NK)]
    wv_a = sb.tile([128, NK * FPC], bf16, name="wva", tag="wva")
    wv_t = [wv_a[:, k * FPC:(k + 1) * FPC] for k in range(NK)]
    wp_a = sb.tile([128, 2 * D], bf16, name="wpa", tag="wpa")
    wp_t = [wp_a[:, k * D:(k + 1) * D] for k in range(2)]
    qt_t = [sb.tile([128, S], bf16, name=f"qtt{f}", tag=f"qtt{f}") for f in range(2)]
    kt_t = [sb.tile([128, S], bf16, name=f"ktt{f}", tag=f"ktt{f}") for f in range(2)]
    v_t = [sb.tile([128, 4 * 65], bf16, name=f"vt{s}", tag=f"vt{s}") for s in range(NS)]
    ot_t = [sb.tile([128, S], bf16, name=f"ott{f}", tag=f"ott{f}") for f in range(2)]
    bqk_t = sb.tile([128, 4], f32, name="bqkt", tag="bqkt")
    mask_t = sb.tile([128, 128], bf16, name="maskt", tag="maskt")
    ones_t = sb.tile([1, 64], f32, name="ones1", tag="ones1")
    nc.vector.memset(ones_t[:], 1.0)

    p_pool = ctx.enter_context(tc.tile_pool(name="pp", bufs=1))
    vs_pool = ctx.enter_context(tc.tile_pool(name="vsp", bufs=2))
    rc_pool = ctx.enter_context(tc.tile_pool(name="rcp", bufs=4))
    oo_pool = ctx.enter_context(tc.tile_pool(name="oop", bufs=2))

    # ---- input DMAs split across four issue queues; each queue's order puts
    # the first attention pair's dependencies first.
    xt3 = xt_a.rearrange("p (k s) -> p k s", k=NK)
    xs3 = xT.rearrange("(k p) s -> p k s", p=128)
    # DMA issue is only possible from SP (sync), Activation (scalar), gpsimd.
    # The critical path for the first matmuls (wq, then the x first-half
    # k-chunks in arrival order) rides the fastest-starting queue (sync).
    # gpsimd's DGE pays a ~6us ucode load, so it only gets late-needed data.
    # The critical prefix is split across queues, and x arrives in s-major
    # 512-column chunks so the sp-major qkt matmuls chase the stream.
    nc.sync.dma_start(wq_a.rearrange("p (k f) -> p k f", k=NK),
                      wq.rearrange("(k p) f -> p k f", p=128))
    nc.sync.dma_start(bqk_t[:], bqk[:])
    nc.sync.dma_start(xt3[:, :, 0:512], xs3[:, :, 0:512])
    nc.sync.dma_start(xt3[:, :, 512:1024], xs3[:, :, 512:1024])
    # scalar: wk (2nd qkt group), mask (first exp), then second-half x
    nc.scalar.dma_start(wk_a.rearrange("p (k f) -> p k f", k=NK),
                        wk.rearrange("(k p) f -> p k f", p=128))
    nc.scalar.dma_start(mask_t[:], maskT[:])
    nc.scalar.dma_start(xt3[:, :, 1024:1536], xs3[:, :, 1024:1536])
    nc.scalar.dma_start(xt3[:, :, 1536:2048], xs3[:, :, 1536:2048])
    # gpsimd (DGE starts ~6us late): wv (prelude v groups), wp (proj)
    nc.gpsimd.dma_start(wv_a.rearrange("p (k f) -> p k f", k=NK),
                        wv.rearrange("(k p) f -> p k f", p=128))
    nc.gpsimd.dma_start(wp_a.rearrange("p (k f) -> p k f", k=2),
                        wp.rearrange("(k p) f -> p k f", p=128))

    # PSUM: "sc" ring (scores + all filler groups) 2 x [128,1024] = 4 banks,
    # "pv" accumulators 2 x [128,1024] = 4 banks.
    scp = ctx.enter_context(tc.tile_pool(name="ps_sc", bufs=2, space="PSUM"))
    pvp = ctx.enter_context(tc.tile_pool(name="ps_pv", bufs=2, space="PSUM"))

    def qkt_group(dst, w_t, bcol, f, c2):
        """One [128,1024] accumulation group of the Q^T/K^T projection.
        sp-major so the first 512 columns complete (and drain) while the
        next x s-chunk is still arriving."""
        ps = scp.tile([128, 1024], f32, name="sc", tag="sc", bufs=2)
        for sp in range(2):
            for k in range(NK):
                nc.tensor.matmul(
                    ps[:, sp * 512:(sp + 1) * 512],
                    w_t[k][:, f * 128:(f + 1) * 128],
                    xt_t[k][:, c2 * 1024 + sp * 512: c2 * 1024 + (sp + 1) * 512],
                    start=(k == 0), stop=(k == NK - 1),
                )
            nc.vector.tensor_scalar_add(
                dst[f][:, c2 * 1024 + sp * 512: c2 * 1024 + (sp + 1) * 512],
                ps[:, sp * 512:(sp + 1) * 512],
                bqk_t[:, bcol + f: bcol + f + 1],
            )

    def v_group(s):
        psv = scp.tile([128, FPC], f32, name="sc", tag="sc", bufs=2)
        for k in range(NK):
            nc.tensor.matmul(
                psv[:],
                xt_t[k][:, s * 128:(s + 1) * 128],
                wv_t[k][:],
                start=(k == 0), stop=(k == NK - 1),
            )
        # NOTE: bulk work on gpsimd (Q7) triggers the power throttle that
        # halves PE duty -- keep these on DVE.
        v3 = v_t[s].rearrange("p (h c) -> p h c", h=4)
        nc.vector.tensor_copy(v3[:, :, 0:64],
                              psv.rearrange("p (h c) -> p h c", h=4)[:])
        nc.vector.memset(v3[:, :, 64:65], 1.0)

    class AttnUnit:
        """Causal attention for head h over queries [half*1024, +1024)."""

        def __init__(self, h, half, dcp_on_act=False, fine_tail=False):
            self.h, self.half = h, half
            self.hp, self.hh = h // 2, h % 2
            self.r0 = self.hh * 64
            self.q0 = half * 1024
            self.ki_n = NS // 2 * (half + 1)
            self.dcp_on_act = dcp_on_act
            # pv accumulation regions are bank-aligned (512 cols); two
            # regions must never share a psum bank while one still
            # accumulates. fine_tail only splits the *finish* of the last
            # bank into 256-col chunks (safe: emitted after the full stop)
            # so the first projection of those columns starts sooner.
            self.pv_bounds = [0, 512, 1024]
            kf0 = min(self.ki_n - 1, (self.q0 + 511) // 128)
            last = self.ki_n - 1
            if fine_tail:
                self.fin = [(0, 512, kf0), (512, 768, last), (768, 1024, last)]
            else:
                self.fin = [(0, 512, kf0), (512, 1024, last)]
            self.pv = pvp.tile([128, 1024], f32, name="pv", tag="pv", bufs=2)
            self.P = {}
            self.spans = {}

        def emit_scores(self, ki):
            q0, r0 = self.q0, self.r0
            qt, kt = qt_t[self.hp], kt_t[self.hp]
            qs = max(ki * 128, q0)   # first unmasked q for this k block
            a0 = qs - q0             # local col offset in the 1024 tile
            diag = ki * 128 >= q0    # diagonal block lives in this half
            spans = [(a0, 512), (512, 1024)] if a0 < 512 else [(a0, 1024)]
            self.spans[ki] = (a0, diag)
            sc = scp.tile([128, 1024], f32, name="sc", tag="sc", bufs=2)
            for (a, b) in spans:
                nc.tensor.matmul(
                    sc[:, a:b],
                    kt[r0:r0 + 64, ki * 128:(ki + 1) * 128],
                    qt[r0:r0 + 64, q0 + a:q0 + b],
                    start=True, stop=True,
                )
            self.sc = sc

        def emit_exp(self, ki):
            a0, diag = self.spans[ki]
            P = p_pool.tile([128, 1024], bf16, name="P", tag="P", bufs=8)
            nc.scalar.activation(P[:, a0:1024], self.sc[:, a0:1024], AF.Exp,
                                 scale=float(HD) ** -0.5)
            if diag:  # causal mask on the diagonal block
                nc.vector.tensor_mul(P[:, a0:a0 + 128],
                                     P[:, a0:a0 + 128], mask_t[:])
            self.P[ki] = P

        def emit_pv(self, ki):
            a0, _ = self.spans[ki]
            P = self.P.pop(ki)
            # subdivide [a0,1024) at the pv region bounds
            for i in range(len(self.pv_bounds) - 1):
                a = max(a0, self.pv_bounds[i])
                b = self.pv_bounds[i + 1]
                if a >= b:
                    continue
                last_ki = min(self.ki_n - 1, (self.q0 + b - 1) // 128)
                nc.tensor.matmul(
                    self.pv[0:65, a:b],
                    v_t[ki][:, self.h * 65:self.h * 65 + 65],
                    P[:, a:b],
                    start=(ki == 0), stop=(ki == last_ki),
                )

        def finish_a(self, a, b):
            """Phase A of the softmax divide: denominator row -> reciprocal
            (DVE) -> partition broadcast (gpsimd). The returned rbc tile is
            consumed by phase B one iteration later, so the in-order DVE
            stream never blocks waiting on the cross-engine hop."""
            pv = self.pv
            w = b - a
            dcp = rc_pool.tile([1, 512], f32, name="dcp", tag="dcp", bufs=4)
            if self.dcp_on_act:
                nc.scalar.copy(dcp[:, 0:w], pv[64:65, a:b])
            else:
                nc.vector.tensor_copy(dcp[:, 0:w], pv[64:65, a:b])
            rcp = rc_pool.tile([1, 512], f32, name="rcp", tag="rcp", bufs=4)
            nc.vector.reciprocal_approx_fast(rcp[:, 0:w], dcp[:, 0:w])
            rbc = rc_pool.tile([64, 512], f32, name="rbc", tag="rbc", bufs=4)
            nc.gpsimd.partition_broadcast(rbc[:, 0:w], rcp[:, 0:w], channels=64)
            return rbc

        def finish_b(self, a, b, rbc):
            """Phase B: scale pv into ot."""
            pv = self.pv
            w = b - a
            nc.vector.tensor_mul(
                ot_t[self.hp][self.r0:self.r0 + 64, self.q0 + a:self.q0 + b],
                pv[0:64, a:b], rbc[:, 0:w],
            )

    def attn_pair(ha, hb, half, fillers=(), dcp_on_act=False, fine_tail=False):
        """Two heads, software-pipelined: scores(t) and pv(t-1) per iteration
        so the exp->mask latency is hidden. fillers[t] is a list of thunks
        emitting independent PE work at the end of iteration t."""
        ua = AttnUnit(ha, half, dcp_on_act, fine_tail)
        ub = AttnUnit(hb, half, dcp_on_act, fine_tail)
        n = ua.ki_n
        # two-iteration lag between scores and pv: the exp (ACT) -> mask
        # (DVE) chain gets ~2 iterations of slack, so DVE bursts (finish
        # spans) cannot stall the PE's pv matmuls. Finishes run in two
        # phases one iteration apart so the broadcast matmul never waits
        # on the reciprocal.
        pending = []
        for t in range(n + 2):
            for (u, a, b, rcp) in pending:
                u.finish_b(a, b, rcp)
            pending = []
            if t < n:
                ua.emit_scores(t)
                ub.emit_scores(t)
                ua.emit_exp(t)
                ub.emit_exp(t)
            if t >= 2:
                ua.emit_pv(t - 2)
                ub.emit_pv(t - 2)
                for (a, b, kf) in ua.fin:
                    if t - 2 == kf:
                        pending.append((ua, a, b, ua.finish_a(a, b)))
                        pending.append((ub, a, b, ub.finish_a(a, b)))
            if t < len(fillers):
                for fn in fillers[t]:
                    fn()
        for (u, a, b, rcp) in pending:
            u.finish_b(a, b, rcp)

    def proj_group(s):
        pj = scp.tile([128, 1024], f32, name="sc", tag="sc", bufs=2)
        for nh in range(2):
            for k2 in range(2):
                nc.tensor.matmul(
                    pj[:, nh * 512:(nh + 1) * 512],
                    ot_t[k2][:, s * 128:(s + 1) * 128],
                    wp_t[k2][:, nh * 512:(nh + 1) * 512],
                    start=(k2 == 0), stop=(k2 == 1),
                )
        oo = oo_pool.tile([128, D], bf16, name="oo", tag="oo", bufs=3)
        # drain on ACT only in the true tail where it has gone idle; putting
        # psum-dependent drains on ACT mid-kernel blocks the in-order exp
        # stream and stalls the whole attention pipeline
        if s >= 12:
            nc.scalar.copy(oo[:], pj[:])
        else:
            nc.vector.tensor_copy(oo[:], pj[:])
        # out DMAs all ride sync: the gpsimd queue is clogged by the final
        # partition_broadcasts right when the last groups drain
        nc.sync.dma_start(out[s * 128:(s + 1) * 128, :], oo[:])

    from functools import partial

    def qkt_half(dst, w_t, bcol, f, c2, sp):
        """512-column half of a qkt group -- prelude granularity that chases
        the arriving x s-chunks."""
        ps = scp.tile([128, 512], f32, name="sc", tag="sc", bufs=2)
        for k in range(NK):
            nc.tensor.matmul(
                ps[:],
                w_t[k][:, f * 128:(f + 1) * 128],
                xt_t[k][:, c2 * 1024 + sp * 512: c2 * 1024 + (sp + 1) * 512],
                start=(k == 0), stop=(k == NK - 1),
            )
        nc.vector.tensor_scalar_add(
            dst[f][:, c2 * 1024 + sp * 512: c2 * 1024 + (sp + 1) * 512],
            ps[:], bqk_t[:, bcol + f: bcol + f + 1],
        )

    # Prelude: exactly what pair (0,1,0) needs to start, in x-arrival order.
    qkt_half(qt_t, wq_t, 0, 0, 0, 0)
    qkt_half(kt_t, wk_t, 2, 0, 0, 0)
    v_group(0)
    qkt_half(qt_t, wq_t, 0, 0, 0, 1)
    qkt_half(kt_t, wk_t, 2, 0, 0, 1)
    v_group(1)

    attn_pair(0, 1, 0, fillers=[
        [partial(v_group, 2)],
        [partial(v_group, 3)],
        [partial(v_group, 4)],
        [partial(v_group, 5)],
        [partial(v_group, 6)],
        [partial(v_group, 7)],
        [partial(qkt_group, qt_t, wq_t, 0, 1, 0)],
        [partial(qkt_group, kt_t, wk_t, 2, 1, 0)],
        [],
    ])
    attn_pair(2, 3, 0, fillers=[
        [partial(qkt_group, qt_t, wq_t, 0, 0, 1)],
        [partial(qkt_group, kt_t, wk_t, 2, 0, 1)],
        [partial(v_group, 8)],
        [],
        [partial(v_group, 9)],
        [],
        [partial(v_group, 10)],
        [],
        [],
    ])
    attn_pair(0, 1, 1, fillers=[
        [partial(qkt_group, qt_t, wq_t, 0, 1, 1)],
        [],
        [partial(qkt_group, kt_t, wk_t, 2, 1, 1)],
        [],
        [partial(v_group, 11)],
        [],
        [partial(v_group, 12)],
        [],
        [partial(v_group, 13)],
        [partial(v_group, 14)],
        [partial(v_group, 15)],
        [], [], [],
        [partial(proj_group, 0)],
        [],
        [partial(proj_group, 1)],
    ])
    attn_pair(2, 3, 1, dcp_on_act=True, fine_tail=True, fillers=[
        [partial(proj_group, 2)],
        [],
        [partial(proj_group, 3)],
        [],
        [partial(proj_group, 4)],
        [],
        [partial(proj_group, 5)],
        [],
        [partial(proj_group, 6)],
        [],
        [partial(proj_group, 7)],
        [], [], [],
        [partial(proj_group, 8)],
        [partial(proj_group, 9)],
        [partial(proj_group, 10)],
        [partial(proj_group, 11)],
    ])
    for s in range(12, NS):
        proj_group(s)


def _in_maps(x, W_qkv, b_qkv, W_proj):
    bf = ml_dtypes.bfloat16
    maps = []
    # multiplicative causal mask for the transposed diag block: keep k<=q
    mask = np.triu(np.ones((128, 128), np.float32)).astype(bf)
    for core in range(NCORES):
        b, hg = core // 4, core % 4
        cs = slice(hg * FPC, (hg + 1) * FPC)
        bq = b_qkv[cs].astype(np.float32)
        bk = b_qkv[D + hg * FPC: D + (hg + 1) * FPC].astype(np.float32)
        maps.append({
            "xT": np.ascontiguousarray(x[b].T).astype(bf),
            "wq": np.ascontiguousarray(W_qkv[:, cs]).astype(bf),
            "wk": np.ascontiguousarray(W_qkv[:, D + hg * FPC: D + (hg + 1) * FPC]).astype(bf),
            "wv": np.ascontiguousarray(W_qkv[:, 2 * D + hg * FPC: 2 * D + (hg + 1) * FPC]).astype(bf),
            "wp": np.ascontiguousarray(W_proj[hg * FPC:(hg + 1) * FPC, :]).astype(bf),
            "bqk": np.ascontiguousarray(
                np.stack([bq[0:128], bq[128:256], bk[0:128], bk[128:256]], axis=1)),
            "maskT": mask,
        })
    return maps


def get_nc():
    if "nc" not in _CACHE:
        _CACHE["nc"] = _build()
    return _CACHE["nc"]


def _postprocess(partials, b_qkv, W_proj, b_proj):
    out = np.zeros((B, S, D), np.float32)
    for core in range(NCORES):
        out[core // 4] += np.asarray(partials[core], np.float32)
    bv = np.asarray(b_qkv, np.float32)[2 * D:3 * D]
    out += bv @ np.asarray(W_proj, np.float32) + np.asarray(b_proj, np.float32)
    return out


def kernel(x, W_qkv, b_qkv, W_proj, b_proj, _trace=False):
    from concourse.bass_utils import run_bass_kernel_spmd

    x = np.asarray(x, np.float32)
    W_qkv = np.asarray(W_qkv, np.float32)
    b_qkv = np.asarray(b_qkv, np.float32)
    W_proj = np.asarray(W_proj, np.float32)
    b_proj = np.asarray(b_proj, np.float32)

    nc = get_nc()
    maps = _in_maps(x, W_qkv, b_qkv, W_proj)
    res = run_bass_kernel_spmd(nc, maps, list(range(NCORES)), trace=_trace)
    _CACHE["last_result"] = res
    partials = [res.results[c]["out"] for c in range(NCORES)]
    return _postprocess(partials, b_qkv, W_proj, b_proj)
